# revision 20
# baseline (speedup 1.0000x reference)
"""Multi-head attention (B=8,T=512,S=1024,E=512,H=8) on 8 TRN2 NeuronCores.

Data-parallel over batch: core b computes batch element b end-to-end.
bf16 TensorEngine matmuls (fp32 PSUM), exp on ScalarEngine, normalization on
VectorEngine, softmax-sum reciprocal broadcast via GpSimd partition_broadcast.

Per-core dataflow:
  qT = Wq @ q_b.T            (E on partitions, T free)
  kT = Wk @ k_b.T            (E on partitions, S free)
  v_ext = [v_b @ Wv.T | 1]   (S on partitions, per-head [64|1] columns)
  scoresT_h = k_h @ q_h.T    (S on partitions, T free)  -> exp (bf16)
  [ctxT'_h ; sums] = [v_h|1].T @ expT_h   (one accumulated matmul)
  weights[h,s,t] = expT_h * bcast(1/sums) (bf16; host transposes to (t,s))
  out = sum_h ctxT_h.T @ WoT_h            (f32)

Emission is hand-interleaved so the TensorEngine's in-order stream never
head-of-line blocks on the ScalarEngine's exp (keeps PE HAM-warm).
"""

import sys
import types

import numpy as np


def _install_ntff_hook():
    if "antenv.axon_hooks" in sys.modules:
        return
    try:
        from trn_agent_boot.trn_boot import _ntff_profile_via_ctypes

        hook = _ntff_profile_via_ctypes("/opt/axon/libaxon_pjrt.so")
    except Exception:
        hook = None
    mod = types.ModuleType("antenv.axon_hooks")
    mod.get_axon_ntff_profile_hook = lambda: hook
    mod.set_axon_ntff_profile_hook = lambda h: None
    sys.modules["antenv.axon_hooks"] = mod


_install_ntff_hook()

import ml_dtypes  # noqa: E402
import concourse.bass as bass  # noqa: E402
import concourse.tile as tile  # noqa: E402
from concourse import bacc, mybir  # noqa: E402
from concourse.bass_utils import run_bass_kernel_spmd  # noqa: E402

B, T, S, E, H = 8, 512, 1024, 512, 8
D = E // H  # 64
P = 128
BF16 = mybir.dt.bfloat16
F32 = mybir.dt.float32
NPBF16 = ml_dtypes.bfloat16

FO = E // P  # 4 contraction chunks
TM = T // P  # 4 query tiles
SM = S // P  # 8 key tiles

_CACHED_NC = None


DEFAULT_CFG = {
    "lag_hi": 2,        # pending-queue depth kept during pairs 1..2
    "warmers": 4,       # PE warm-keeper matmuls in the drain
    "psum_s_bufs": 2,
    "psum_c_bufs": 2,
    "psum_p_bufs": 2,
    "expp_bufs": 6,
    "wout_bufs": 4,
    "v_evac_act": False,
    "qk_evac_act": False,
    "sums_act": False,
}


def _build_nc(cfg=None):
    cfg = {**DEFAULT_CFG, **(cfg or {})}
    nc = bacc.Bacc("TRN2", target_bir_lowering=False, debug=False, num_devices=1)

    qT_in = nc.declare_dram_parameter("qT_in", [E, T], BF16, isOutput=False)
    kT_in = nc.declare_dram_parameter("kT_in", [E, S], BF16, isOutput=False)
    vT_in = nc.declare_dram_parameter("vT_in", [E, S], BF16, isOutput=False)
    wqT = nc.declare_dram_parameter("wqT", [E, E], BF16, isOutput=False)
    wkT = nc.declare_dram_parameter("wkT", [E, E], BF16, isOutput=False)
    wvT = nc.declare_dram_parameter("wvT", [E, E], BF16, isOutput=False)
    woT = nc.declare_dram_parameter("woT", [E, E], BF16, isOutput=False)
    out_ext = nc.declare_dram_parameter("out", [T, E], F32, isOutput=True)
    w_ext = nc.declare_dram_parameter("w", [H, S, T], BF16, isOutput=True)

    with tile.TileContext(nc, pool_alloc_mode="queue") as tc:
        with (
            tc.tile_pool(name="stage", bufs=1) as stage,
            tc.tile_pool(name="proj", bufs=1) as proj,
            tc.tile_pool(name="expp", bufs=cfg["expp_bufs"]) as expp,
            tc.tile_pool(name="wout", bufs=cfg["wout_bufs"]) as wout,
            tc.tile_pool(name="small", bufs=4) as small,
            tc.tile_pool(name="psum_p", bufs=cfg["psum_p_bufs"], space="PSUM") as psum_p,
            tc.tile_pool(name="psum_s", bufs=cfg["psum_s_bufs"], space="PSUM") as psum_s,
            tc.tile_pool(name="psum_c", bufs=cfg["psum_c_bufs"], space="PSUM") as psum_c,
        ):
            # ---- staged inputs (DMA'd in dependency-first order, per chunk) ----
            qt_s = stage.tile([P, FO, T], BF16, tag="qt_in")
            kt_s = stage.tile([P, FO, S], BF16, tag="kt_in")
            vt_s = stage.tile([P, FO, S], BF16, tag="vt_in")
            wq_s = stage.tile([P, FO, E], BF16, tag="wq")
            wk_s = stage.tile([P, FO, E], BF16, tag="wk")
            wv_s = stage.tile([P, FO, E], BF16, tag="wv")
            wo_s = stage.tile([P, FO, E], BF16, tag="wo")
            qt_d = qT_in.rearrange("(fo fi) t -> fi fo t", fi=P)
            kt_d = kT_in.rearrange("(fo fi) s -> fi fo s", fi=P)
            vt_d = vT_in.rearrange("(fo fi) s -> fi fo s", fi=P)
            wq_d = wqT.rearrange("(fo fi) e -> fi fo e", fi=P)
            wk_d = wkT.rearrange("(fo fi) e -> fi fo e", fi=P)
            wv_d = wvT.rearrange("(fo fi) e -> fi fo e", fi=P)
            # critical-path inputs first, alternating issue between SP and ACT;
            # only the m=0 column block of Wq/Wk gates the first score matmuls
            for ko in range(FO):
                nc.sync.dma_start(wq_s[:, ko, 0:P], wq_d[:, ko, 0:P])
                nc.scalar.dma_start(qt_s[:, ko], qt_d[:, ko])
                nc.sync.dma_start(wk_s[:, ko, 0:P], wk_d[:, ko, 0:P])
                nc.scalar.dma_start(kt_s[:, ko, 0:512], kt_d[:, ko, 0:512])
            for ko in range(FO):
                nc.sync.dma_start(wq_s[:, ko, P:E], wq_d[:, ko, P:E])
                nc.scalar.dma_start(wk_s[:, ko, P:E], wk_d[:, ko, P:E])
            for ko in range(FO):
                nc.sync.dma_start(kt_s[:, ko, 512:1024], kt_d[:, ko, 512:1024])
                nc.scalar.dma_start(wv_s[:, ko], wv_d[:, ko])
                nc.sync.dma_start(vt_s[:, ko, 0:512], vt_d[:, ko, 0:512])
                nc.scalar.dma_start(vt_s[:, ko, 512:1024], vt_d[:, ko, 512:1024])
            nc.gpsimd.dma_start(wo_s[:], woT.rearrange("(fo fi) e -> fi fo e", fi=P))

            qT = [proj.tile([P, T], BF16, tag=f"qT{m}", name=f"qT{m}") for m in range(FO)]
            kT = [proj.tile([P, S], BF16, tag=f"kT{m}", name=f"kT{m}") for m in range(FO)]
            v_ext = [
                proj.tile([P, H, D + 1], BF16, tag=f"vx{sm}", name=f"vx{sm}")
                for sm in range(SM)
            ]
            ctxT = [
                proj.tile([P, T], BF16, tag=f"ctxT{pj}", name=f"ctxT{pj}")
                for pj in range(H // 2)
            ]
            ones_sb = small.tile([1, P], F32, tag="ones")
            nc.vector.memset(ones_sb[:], 1.0)
            warm_src = small.tile([1, 512], F32, tag="warmsrc")
            nc.vector.memset(warm_src[:], 0.0)

            def emit_proj_q(m):
                ps_q = psum_p.tile([P, 512], F32, tag="ps_p", name="ps_q")
                for ko in range(FO):
                    nc.tensor.matmul(
                        ps_q[:], wq_s[:, ko, m * P : (m + 1) * P], qt_s[:, ko, :],
                        start=(ko == 0), stop=(ko == FO - 1),
                    )
                if cfg['qk_evac_act']:
                    nc.scalar.copy(qT[m][:], ps_q[:])
                else:
                    nc.vector.tensor_copy(qT[m][:], ps_q[:])

            def emit_proj_k(m, sh):
                ps_k = psum_p.tile([P, 512], F32, tag="ps_p", name="ps_k")
                for ko in range(FO):
                    nc.tensor.matmul(
                        ps_k[:], wk_s[:, ko, m * P : (m + 1) * P],
                        kt_s[:, ko, sh * 512 : (sh + 1) * 512],
                        start=(ko == 0), stop=(ko == FO - 1),
                    )
                if cfg['qk_evac_act']:
                    nc.scalar.copy(kT[m][:, sh * 512 : (sh + 1) * 512], ps_k[:])
                else:
                    nc.vector.tensor_copy(kT[m][:, sh * 512 : (sh + 1) * 512], ps_k[:])

            def emit_proj_qk(m):
                emit_proj_q(m)
                emit_proj_k(m, 0)
                emit_proj_k(m, 1)

            def emit_proj_v(sm):
                ps_v = psum_p.tile([P, 512], F32, tag="ps_p", name="ps_v")
                for ko in range(FO):
                    nc.tensor.matmul(
                        ps_v[:], vt_s[:, ko, sm * P : (sm + 1) * P], wv_s[:, ko, :],
                        start=(ko == 0), stop=(ko == FO - 1),
                    )
                if cfg["v_evac_act"]:
                    nc.scalar.copy(
                        v_ext[sm][:, :, 0:D], ps_v[:].rearrange("p (h d) -> p h d", h=H)
                    )
                else:
                    nc.vector.tensor_copy(
                        v_ext[sm][:, :, 0:D], ps_v[:].rearrange("p (h d) -> p h d", h=H)
                    )
                nc.vector.memset(v_ext[sm][:, :, D : D + 1], 1.0)

            # scores+exp for one sg (two s-tiles) of head pair j.
            # MMs interleave the two heads so adjacent matmuls hit different
            # PE row groups (K=64 at partition 0 vs 64) and run concurrently.
            def emit_scores_sg(j, sg, exps):
                m = j
                pss = [
                    psum_s.tile([P, 1024], F32, tag="ps_s", name=f"ps_s{j}{sg}{par}")
                    for par in range(2)
                ]
                for half in range(2):
                    st = sg * 2 + half
                    for par, off in ((0, 0), (1, D)):
                        nc.tensor.matmul(
                            pss[par][:, half * 512 : (half + 1) * 512],
                            kT[m][off : off + D, st * P : (st + 1) * P],
                            qT[m][off : off + D, :],
                        )
                for par in range(2):
                    nc.scalar.activation(
                        exps[par][:, sg * 1024 : (sg + 1) * 1024],
                        pss[par][:],
                        mybir.ActivationFunctionType.Exp,
                        scale=float(1.0 / np.sqrt(D)),
                    )

            # ctx accumulation + 1/sums + broadcast for one head
            def emit_ctx(h, exps_h):
                ps_c = psum_c.tile([D + 1, T], F32, tag="ps_c", name=f"ps_c{h}")
                for st in range(SM):
                    nc.tensor.matmul(
                        ps_c[:],
                        v_ext[st][:, h, :],
                        exps_h[:, st * 512 : (st + 1) * 512],
                        start=(st == 0), stop=(st == SM - 1),
                    )
                sums = small.tile([1, T], F32, tag="sums", name=f"su{h}")
                if cfg['sums_act']:
                    nc.scalar.copy(sums[:], ps_c[D : D + 1, :])
                else:
                    nc.vector.tensor_copy(sums[:], ps_c[D : D + 1, :])
                rsum = small.tile([1, T], F32, tag="rsum", name=f"rs{h}")
                nc.vector.reciprocal_approx_fast(rsum[:], sums[:])
                ps_b = psum_p.tile([P, 512], F32, tag="ps_p", name=f"pb{h}")
                nc.tensor.matmul(ps_b[:, 0:T], ones_sb[:], rsum[:])
                bcast = small.tile([P, T], BF16, tag="bcast", name=f"bc{h}")
                nc.vector.tensor_copy(bcast[:], ps_b[:, 0:T])
                return ps_c, bcast

            def emit_norm_ctx(h, ps_c, bcast):
                off = (h % 2) * D
                nc.vector.tensor_tensor(
                    ctxT[h // 2][off : off + D, :],
                    ps_c[0:D, :],
                    bcast[0:D, :],
                    mybir.AluOpType.mult,
                )

            def emit_norm_w(h, exps_h, bcast):
                hh = SM // 2
                for half in range(2):
                    w_sb = wout.tile([P, hh, T], BF16, tag="w_out", name=f"w{h}{half}")
                    nc.vector.tensor_tensor(
                        w_sb[:],
                        exps_h[:, half * hh * T : (half + 1) * hh * T].rearrange(
                            "p (a t) -> p a t", a=hh
                        ),
                        bcast[:, None, :].to_broadcast((P, hh, T)),
                        mybir.AluOpType.mult,
                    )
                    for sg in range(2):
                        g = half * 2 + sg
                        nc.sync.dma_start(
                            w_ext[h, g * 2 * P : (g * 2 + 2) * P, :].rearrange(
                                "(o p) t -> p o t", p=P
                            ),
                            w_sb[:, sg * 2 : sg * 2 + 2, :],
                        )

            def emit_final(tm):
                ps_o = psum_p.tile([P, 512], F32, tag="ps_p", name=f"ps_o{tm}")
                for pj in range(H // 2):
                    nc.tensor.matmul(
                        ps_o[:], ctxT[pj][:, tm * P : (tm + 1) * P], wo_s[:, pj, :],
                        start=(pj == 0), stop=(pj == H // 2 - 1),
                    )
                o_sb = wout.tile([P, E], F32, tag="o_out", name=f"o{tm}")
                nc.scalar.copy(o_sb[:], ps_o[:])
                nc.sync.dma_start(out_ext[tm * P : (tm + 1) * P, :], o_sb[:])

            # tail of head pair j as 4 chunks (interleaved between sg groups)
            def tail_chunks(j, exps):
                he, ho = 2 * j, 2 * j + 1
                st8 = {}

                def c0():
                    st8["e"] = emit_ctx(he, exps[0])

                def c1():
                    st8["o"] = emit_ctx(ho, exps[1])

                def c2():
                    ps_e, bc_e = st8["e"]
                    emit_norm_ctx(he, ps_e, bc_e)
                    emit_norm_w(he, exps[0], bc_e)

                def c3():
                    ps_o_, bc_o = st8["o"]
                    emit_norm_ctx(ho, ps_o_, bc_o)
                    emit_norm_w(ho, exps[1], bc_o)

                return [c0, c1, c2, c3]

            pair_exps = {}

            # ---------- hand-interleaved emission schedule ----------
            emit_proj_q(0)
            emit_proj_k(0, 0)

            def alloc_exps(j):
                return (
                    expp.tile([P, 4096], BF16, tag="expT", name=f"ex{j}e"),
                    expp.tile([P, 4096], BF16, tag="expT", name=f"ex{j}o"),
                )

            # pair 0 scores interleaved with the rest of the projections
            pair_exps[0] = alloc_exps(0)
            emit_scores_sg(0, 0, pair_exps[0])
            emit_proj_v(0)
            emit_scores_sg(0, 1, pair_exps[0])
            emit_proj_k(0, 1)
            emit_proj_v(1)
            emit_scores_sg(0, 2, pair_exps[0])
            emit_proj_v(2)
            emit_proj_v(3)
            emit_scores_sg(0, 3, pair_exps[0])
            for sm in range(4, SM):
                emit_proj_v(sm)

            # tail chunks are consumed from a queue lagging the scores by two
            # slots, so the TensorEngine always has near-term work and never
            # idles long enough to re-throttle (HAM MID window).
            from collections import deque

            pending = deque()
            pending.extend(tail_chunks(0, pair_exps[0]))

            for j in range(1, 4):
                emit_proj_qk(j)
                pair_exps[j] = alloc_exps(j)
                for sg in range(4):
                    emit_scores_sg(j, sg, pair_exps[j])
                    if len(pending) > cfg["lag_hi"] or (j == 3 and pending):
                        pending.popleft()()
                pending.extend(tail_chunks(j, pair_exps[j]))

            # drain: remaining chunks, with the output projection overlapped.
            # A few throwaway rank-1 matmuls keep the PE HAM-warm while the
            # last pair's exps finish (otherwise the idle gap re-throttles the
            # clock and the whole output projection runs at 1.2 GHz).
            while len(pending) > 4:
                pending.popleft()()
            for w in range(cfg["warmers"]):
                ps_w = psum_p.tile([P, 512], F32, tag="ps_p", name=f"warm{w}")
                nc.tensor.matmul(ps_w[:], ones_sb[:], warm_src[:])
            while len(pending) > 2:
                pending.popleft()()
            pending.popleft()()  # c2 of pair 3 (ctxT h6 + weights h6)
            pending.popleft()()  # c3 of pair 3 (ctxT h7 + weights h7)
            for tm in range(TM):
                emit_final(tm)

    nc.finalize()
    return nc


def _get_nc():
    global _CACHED_NC
    if _CACHED_NC is None:
        _CACHED_NC = _build_nc()
    return _CACHED_NC


def _np_reference(queries, keys, values, q_padding_mask, key_padding_mask, attn_mask,
                  Wq, bq, Wk, bk, Wv, bv, Wo, bo):
    """Numpy fallback for non-trivial masks/biases (never hit by the harness)."""
    q = queries @ Wq.T + bq
    k = keys @ Wk.T + bk
    v = values @ Wv.T + bv
    q = q.reshape(B, T, H, D).transpose(0, 2, 1, 3)
    k = k.reshape(B, S, H, D).transpose(0, 2, 1, 3)
    v = v.reshape(B, S, H, D).transpose(0, 2, 1, 3)
    scores = np.einsum("bhtd,bhsd->bhts", q, k) / np.sqrt(D)
    scores = np.where(key_padding_mask[:, None, None, :], -1e30, scores)
    scores = np.where(attn_mask[:, None, :, :], scores, -1e30)
    scores = scores - scores.max(axis=-1, keepdims=True)
    e = np.exp(scores)
    weights = e / e.sum(axis=-1, keepdims=True)
    weights = np.where(q_padding_mask[:, None, :, None], 0.0, weights)
    context = np.einsum("bhts,bhsd->bhtd", weights, v)
    context = context.transpose(0, 2, 1, 3).reshape(B, T, E)
    out = context @ Wo.T + bo
    return out.astype(np.float32), weights.astype(np.float32)


def kernel(queries, keys, values, q_padding_mask, key_padding_mask, attn_mask,
           Wq, bq, Wk, bk, Wv, bv, Wo, bo, _trace=False):
    queries = np.asarray(queries, np.float32)
    keys = np.asarray(keys, np.float32)
    values = np.asarray(values, np.float32)
    q_padding_mask = np.asarray(q_padding_mask)
    key_padding_mask = np.asarray(key_padding_mask)
    attn_mask = np.asarray(attn_mask)
    Wq, bq = np.asarray(Wq, np.float32), np.asarray(bq, np.float32)
    Wk, bk = np.asarray(Wk, np.float32), np.asarray(bk, np.float32)
    Wv, bv = np.asarray(Wv, np.float32), np.asarray(bv, np.float32)
    Wo, bo = np.asarray(Wo, np.float32), np.asarray(bo, np.float32)

    trivial = (
        not q_padding_mask.any()
        and not key_padding_mask.any()
        and attn_mask.all()
        and not bq.any() and not bk.any() and not bv.any() and not bo.any()
    )
    if not trivial:
        return _np_reference(
            queries, keys, values, q_padding_mask, key_padding_mask, attn_mask,
            Wq, bq, Wk, bk, Wv, bv, Wo, bo,
        )

    nc = _get_nc()

    wqT = np.ascontiguousarray(Wq.T).astype(NPBF16)
    wkT = np.ascontiguousarray(Wk.T).astype(NPBF16)
    wvT = np.ascontiguousarray(Wv.T).astype(NPBF16)
    woT = np.ascontiguousarray(Wo.T).astype(NPBF16)
    in_maps = []
    for b in range(B):
        in_maps.append(
            {
                "qT_in": np.ascontiguousarray(queries[b].T).astype(NPBF16),
                "kT_in": np.ascontiguousarray(keys[b].T).astype(NPBF16),
                "vT_in": np.ascontiguousarray(values[b].T).astype(NPBF16),
                "wqT": wqT,
                "wkT": wkT,
                "wvT": wvT,
                "woT": woT,
            }
        )

    res = run_bass_kernel_spmd(nc, in_maps, core_ids=list(range(B)), trace=_trace)
    if _trace and res.exec_time_ns is not None:
        kernel.last_exec_time_ns = res.exec_time_ns

    out = np.stack([res.results[b]["out"] for b in range(B)])  # (B, T, E) f32
    w_dev = np.stack([res.results[b]["w"] for b in range(B)])  # (B, H, S, T) bf16
    weights = np.ascontiguousarray(w_dev.swapaxes(2, 3)).astype(np.float32)
    return out, weights


# revision 24
# speedup vs baseline: 1.0905x; 1.0905x over previous
"""Multi-head attention (B=8,T=512,S=1024,E=512,H=8) on 8 TRN2 NeuronCores.

Data-parallel over batch: core b computes batch element b end-to-end.
bf16 TensorEngine matmuls (fp32 PSUM), exp on ScalarEngine, normalization on
VectorEngine, softmax-sum reciprocal broadcast via GpSimd partition_broadcast.

Per-core dataflow:
  qT = Wq @ q_b.T            (E on partitions, T free)
  kT = Wk @ k_b.T            (E on partitions, S free)
  v_ext = [v_b @ Wv.T | 1]   (S on partitions, per-head [64|1] columns)
  scoresT_h = k_h @ q_h.T    (S on partitions, T free)  -> exp (bf16)
  [ctxT'_h ; sums] = [v_h|1].T @ expT_h   (one accumulated matmul)
  weights[h,s,t] = expT_h * bcast(1/sums) (bf16; host transposes to (t,s))
  out = sum_h ctxT_h.T @ WoT_h            (f32)

Emission is hand-interleaved so the TensorEngine's in-order stream never
head-of-line blocks on the ScalarEngine's exp (keeps PE HAM-warm).
"""

import sys
import types

import numpy as np


def _install_ntff_hook():
    if "antenv.axon_hooks" in sys.modules:
        return
    try:
        from trn_agent_boot.trn_boot import _ntff_profile_via_ctypes

        hook = _ntff_profile_via_ctypes("/opt/axon/libaxon_pjrt.so")
    except Exception:
        hook = None
    mod = types.ModuleType("antenv.axon_hooks")
    mod.get_axon_ntff_profile_hook = lambda: hook
    mod.set_axon_ntff_profile_hook = lambda h: None
    sys.modules["antenv.axon_hooks"] = mod


_install_ntff_hook()

import ml_dtypes  # noqa: E402
import concourse.bass as bass  # noqa: E402
import concourse.tile as tile  # noqa: E402
from concourse import bacc, mybir  # noqa: E402
from concourse.bass_utils import run_bass_kernel_spmd  # noqa: E402

B, T, S, E, H = 8, 512, 1024, 512, 8
D = E // H  # 64
P = 128
BF16 = mybir.dt.bfloat16
F32 = mybir.dt.float32
NPBF16 = ml_dtypes.bfloat16

FO = E // P  # 4 contraction chunks
TM = T // P  # 4 query tiles
SM = S // P  # 8 key tiles

_CACHED_NC = None


DEFAULT_CFG = {
    "lag_hi": 2,        # pending-queue depth kept during pairs 1..2
    "warmers": 4,       # PE warm-keeper matmuls in the drain
    "psum_s_bufs": 2,
    "psum_c_bufs": 2,
    "psum_p_bufs": 2,
    "expp_bufs": 6,
    "wout_bufs": 4,
    "frontload_proj": False,
    "v_evac_act": False,
    "qk_evac_act": False,
    "sums_act": False,
}


def _build_nc(cfg=None):
    cfg = {**DEFAULT_CFG, **(cfg or {})}
    nc = bacc.Bacc("TRN2", target_bir_lowering=False, debug=False, num_devices=1)

    qT_in = nc.declare_dram_parameter("qT_in", [E, T], BF16, isOutput=False)
    kT_in = nc.declare_dram_parameter("kT_in", [E, S], BF16, isOutput=False)
    vT_in = nc.declare_dram_parameter("vT_in", [E, S], BF16, isOutput=False)
    wqT = nc.declare_dram_parameter("wqT", [E, E], BF16, isOutput=False)
    wkT = nc.declare_dram_parameter("wkT", [E, E], BF16, isOutput=False)
    wvT = nc.declare_dram_parameter("wvT", [E, E], BF16, isOutput=False)
    woT = nc.declare_dram_parameter("woT", [E, E], BF16, isOutput=False)
    out_ext = nc.declare_dram_parameter("out", [T, E], F32, isOutput=True)
    w_ext = nc.declare_dram_parameter("w", [H, S, T], BF16, isOutput=True)

    with tile.TileContext(nc) as tc:
        with (
            tc.tile_pool(name="stage", bufs=1) as stage,
            tc.tile_pool(name="proj", bufs=1) as proj,
            tc.tile_pool(name="expp", bufs=cfg["expp_bufs"]) as expp,
            tc.tile_pool(name="wout", bufs=cfg["wout_bufs"]) as wout,
            tc.tile_pool(name="small", bufs=4) as small,
            tc.tile_pool(name="psum_p", bufs=cfg["psum_p_bufs"], space="PSUM") as psum_p,
            tc.tile_pool(name="psum_s", bufs=cfg["psum_s_bufs"], space="PSUM") as psum_s,
            tc.tile_pool(name="psum_c", bufs=cfg["psum_c_bufs"], space="PSUM") as psum_c,
        ):
            # ---- staged inputs (DMA'd in dependency-first order, per chunk) ----
            qt_s = stage.tile([P, FO, T], BF16, tag="qt_in")
            kt_s = stage.tile([P, FO, S], BF16, tag="kt_in")
            vt_s = stage.tile([P, FO, S], BF16, tag="vt_in")
            wq_s = stage.tile([P, FO, E], BF16, tag="wq")
            wk_s = stage.tile([P, FO, E], BF16, tag="wk")
            wv_s = stage.tile([P, FO, E], BF16, tag="wv")
            wo_s = stage.tile([P, FO, E], BF16, tag="wo")
            qt_d = qT_in.rearrange("(fo fi) t -> fi fo t", fi=P)
            kt_d = kT_in.rearrange("(fo fi) s -> fi fo s", fi=P)
            vt_d = vT_in.rearrange("(fo fi) s -> fi fo s", fi=P)
            wq_d = wqT.rearrange("(fo fi) e -> fi fo e", fi=P)
            wk_d = wkT.rearrange("(fo fi) e -> fi fo e", fi=P)
            wv_d = wvT.rearrange("(fo fi) e -> fi fo e", fi=P)
            # critical-path inputs first, alternating issue between SP and ACT
            for ko in range(FO):
                nc.sync.dma_start(wq_s[:, ko], wq_d[:, ko])
                nc.scalar.dma_start(qt_s[:, ko], qt_d[:, ko])
                nc.sync.dma_start(wk_s[:, ko], wk_d[:, ko])
                nc.scalar.dma_start(kt_s[:, ko, 0:512], kt_d[:, ko, 0:512])
            for ko in range(FO):
                nc.sync.dma_start(kt_s[:, ko, 512:1024], kt_d[:, ko, 512:1024])
                nc.scalar.dma_start(wv_s[:, ko], wv_d[:, ko])
                nc.sync.dma_start(vt_s[:, ko, 0:512], vt_d[:, ko, 0:512])
                nc.scalar.dma_start(vt_s[:, ko, 512:1024], vt_d[:, ko, 512:1024])
            nc.gpsimd.dma_start(wo_s[:], woT.rearrange("(fo fi) e -> fi fo e", fi=P))

            qT = [proj.tile([P, T], BF16, tag=f"qT{m}", name=f"qT{m}") for m in range(FO)]
            kT = [proj.tile([P, S], BF16, tag=f"kT{m}", name=f"kT{m}") for m in range(FO)]
            v_ext = [
                proj.tile([P, H, D + 1], BF16, tag=f"vx{sm}", name=f"vx{sm}")
                for sm in range(SM)
            ]
            ctxT = [
                proj.tile([P, T], BF16, tag=f"ctxT{pj}", name=f"ctxT{pj}")
                for pj in range(H // 2)
            ]
            ones_sb = small.tile([1, P], F32, tag="ones")
            nc.vector.memset(ones_sb[:], 1.0)
            warm_src = small.tile([1, 512], F32, tag="warmsrc")
            nc.vector.memset(warm_src[:], 0.0)

            def emit_proj_q(m):
                ps_q = psum_p.tile([P, 512], F32, tag="ps_p", name="ps_q")
                for ko in range(FO):
                    nc.tensor.matmul(
                        ps_q[:], wq_s[:, ko, m * P : (m + 1) * P], qt_s[:, ko, :],
                        start=(ko == 0), stop=(ko == FO - 1),
                    )
                if cfg['qk_evac_act']:
                    nc.scalar.copy(qT[m][:], ps_q[:])
                else:
                    nc.vector.tensor_copy(qT[m][:], ps_q[:])

            def emit_proj_k(m, sh):
                ps_k = psum_p.tile([P, 512], F32, tag="ps_p", name="ps_k")
                for ko in range(FO):
                    nc.tensor.matmul(
                        ps_k[:], wk_s[:, ko, m * P : (m + 1) * P],
                        kt_s[:, ko, sh * 512 : (sh + 1) * 512],
                        start=(ko == 0), stop=(ko == FO - 1),
                    )
                if cfg['qk_evac_act']:
                    nc.scalar.copy(kT[m][:, sh * 512 : (sh + 1) * 512], ps_k[:])
                else:
                    nc.vector.tensor_copy(kT[m][:, sh * 512 : (sh + 1) * 512], ps_k[:])

            def emit_proj_qk(m):
                emit_proj_q(m)
                emit_proj_k(m, 0)
                emit_proj_k(m, 1)

            def emit_proj_v(sm):
                ps_v = psum_p.tile([P, 512], F32, tag="ps_p", name="ps_v")
                for ko in range(FO):
                    nc.tensor.matmul(
                        ps_v[:], vt_s[:, ko, sm * P : (sm + 1) * P], wv_s[:, ko, :],
                        start=(ko == 0), stop=(ko == FO - 1),
                    )
                if cfg["v_evac_act"]:
                    nc.scalar.copy(
                        v_ext[sm][:, :, 0:D], ps_v[:].rearrange("p (h d) -> p h d", h=H)
                    )
                else:
                    nc.vector.tensor_copy(
                        v_ext[sm][:, :, 0:D], ps_v[:].rearrange("p (h d) -> p h d", h=H)
                    )
                nc.vector.memset(v_ext[sm][:, :, D : D + 1], 1.0)

            # scores+exp for one sg (two s-tiles) of head pair j.
            # MMs interleave the two heads so adjacent matmuls hit different
            # PE row groups (K=64 at partition 0 vs 64) and run concurrently.
            def emit_scores_sg(j, sg, exps):
                m = j
                pss = [
                    psum_s.tile([P, 1024], F32, tag="ps_s", name=f"ps_s{j}{sg}{par}")
                    for par in range(2)
                ]
                for half in range(2):
                    st = sg * 2 + half
                    for par, off in ((0, 0), (1, D)):
                        nc.tensor.matmul(
                            pss[par][:, half * 512 : (half + 1) * 512],
                            kT[m][off : off + D, st * P : (st + 1) * P],
                            qT[m][off : off + D, :],
                        )
                for par in range(2):
                    nc.scalar.activation(
                        exps[par][:, sg * 1024 : (sg + 1) * 1024],
                        pss[par][:],
                        mybir.ActivationFunctionType.Exp,
                        scale=float(1.0 / np.sqrt(D)),
                    )

            # ctx accumulation + 1/sums + broadcast for one head
            def emit_ctx(h, exps_h):
                ps_c = psum_c.tile([D + 1, T], F32, tag="ps_c", name=f"ps_c{h}")
                for st in range(SM):
                    nc.tensor.matmul(
                        ps_c[:],
                        v_ext[st][:, h, :],
                        exps_h[:, st * 512 : (st + 1) * 512],
                        start=(st == 0), stop=(st == SM - 1),
                    )
                sums = small.tile([1, T], F32, tag="sums", name=f"su{h}")
                if cfg['sums_act']:
                    nc.scalar.copy(sums[:], ps_c[D : D + 1, :])
                else:
                    nc.vector.tensor_copy(sums[:], ps_c[D : D + 1, :])
                rsum = small.tile([1, T], F32, tag="rsum", name=f"rs{h}")
                nc.vector.reciprocal_approx_fast(rsum[:], sums[:])
                ps_b = psum_p.tile([P, 512], F32, tag="ps_p", name=f"pb{h}")
                nc.tensor.matmul(ps_b[:, 0:T], ones_sb[:], rsum[:])
                bcast = small.tile([P, T], BF16, tag="bcast", name=f"bc{h}")
                nc.vector.tensor_copy(bcast[:], ps_b[:, 0:T])
                return ps_c, bcast

            def emit_norm_ctx(h, ps_c, bcast):
                off = (h % 2) * D
                nc.vector.tensor_tensor(
                    ctxT[h // 2][off : off + D, :],
                    ps_c[0:D, :],
                    bcast[0:D, :],
                    mybir.AluOpType.mult,
                )

            def emit_norm_w(h, exps_h, bcast):
                hh = SM // 2
                for half in range(2):
                    w_sb = wout.tile([P, hh, T], BF16, tag="w_out", name=f"w{h}{half}")
                    nc.vector.tensor_tensor(
                        w_sb[:],
                        exps_h[:, half * hh * T : (half + 1) * hh * T].rearrange(
                            "p (a t) -> p a t", a=hh
                        ),
                        bcast[:, None, :].to_broadcast((P, hh, T)),
                        mybir.AluOpType.mult,
                    )
                    for sg in range(2):
                        g = half * 2 + sg
                        nc.sync.dma_start(
                            w_ext[h, g * 2 * P : (g * 2 + 2) * P, :].rearrange(
                                "(o p) t -> p o t", p=P
                            ),
                            w_sb[:, sg * 2 : sg * 2 + 2, :],
                        )

            def emit_final(tm):
                ps_o = psum_p.tile([P, 512], F32, tag="ps_p", name=f"ps_o{tm}")
                for pj in range(H // 2):
                    nc.tensor.matmul(
                        ps_o[:], ctxT[pj][:, tm * P : (tm + 1) * P], wo_s[:, pj, :],
                        start=(pj == 0), stop=(pj == H // 2 - 1),
                    )
                o_sb = wout.tile([P, E], F32, tag="o_out", name=f"o{tm}")
                nc.scalar.copy(o_sb[:], ps_o[:])
                nc.sync.dma_start(out_ext[tm * P : (tm + 1) * P, :], o_sb[:])

            # tail of head pair j as 4 chunks (interleaved between sg groups)
            def tail_chunks(j, exps):
                he, ho = 2 * j, 2 * j + 1
                st8 = {}

                def c0():
                    st8["e"] = emit_ctx(he, exps[0])

                def c1():
                    st8["o"] = emit_ctx(ho, exps[1])

                def c2():
                    ps_e, bc_e = st8["e"]
                    emit_norm_ctx(he, ps_e, bc_e)
                    emit_norm_w(he, exps[0], bc_e)

                def c3():
                    ps_o_, bc_o = st8["o"]
                    emit_norm_ctx(ho, ps_o_, bc_o)
                    emit_norm_w(ho, exps[1], bc_o)

                return [c0, c1, c2, c3]

            pair_exps = {}

            # ---------- hand-interleaved emission schedule ----------
            emit_proj_q(0)
            emit_proj_k(0, 0)

            def alloc_exps(j):
                return (
                    expp.tile([P, 4096], BF16, tag="expT", name=f"ex{j}e"),
                    expp.tile([P, 4096], BF16, tag="expT", name=f"ex{j}o"),
                )

            # pair 0 scores interleaved with the rest of the projections
            pair_exps[0] = alloc_exps(0)
            emit_scores_sg(0, 0, pair_exps[0])
            emit_proj_v(0)
            emit_scores_sg(0, 1, pair_exps[0])
            emit_proj_k(0, 1)
            emit_proj_v(1)
            emit_scores_sg(0, 2, pair_exps[0])
            emit_proj_v(2)
            emit_proj_v(3)
            emit_scores_sg(0, 3, pair_exps[0])
            for sm in range(4, SM):
                emit_proj_v(sm)

            # tail chunks are consumed from a queue lagging the scores by two
            # slots, so the TensorEngine always has near-term work and never
            # idles long enough to re-throttle (HAM MID window).
            from collections import deque

            pending = deque()
            pending.extend(tail_chunks(0, pair_exps[0]))

            if cfg["frontload_proj"]:
                for m in range(1, 4):
                    emit_proj_qk(m)
            for j in range(1, 4):
                if not cfg["frontload_proj"]:
                    emit_proj_qk(j)
                pair_exps[j] = alloc_exps(j)
                for sg in range(4):
                    emit_scores_sg(j, sg, pair_exps[j])
                    if len(pending) > cfg["lag_hi"] or (j == 3 and pending):
                        pending.popleft()()
                pending.extend(tail_chunks(j, pair_exps[j]))

            # drain: remaining chunks, with the output projection overlapped.
            # A few throwaway rank-1 matmuls keep the PE HAM-warm while the
            # last pair's exps finish (otherwise the idle gap re-throttles the
            # clock and the whole output projection runs at 1.2 GHz).
            while len(pending) > 4:
                pending.popleft()()
            for w in range(cfg["warmers"]):
                ps_w = psum_p.tile([P, 512], F32, tag="ps_p", name=f"warm{w}")
                nc.tensor.matmul(ps_w[:], ones_sb[:], warm_src[:])
            while len(pending) > 2:
                pending.popleft()()
            pending.popleft()()  # c2 of pair 3 (ctxT h6 + weights h6)
            pending.popleft()()  # c3 of pair 3 (ctxT h7 + weights h7)
            for tm in range(TM):
                emit_final(tm)

    nc.finalize()
    return nc


def _get_nc():
    global _CACHED_NC
    if _CACHED_NC is None:
        _CACHED_NC = _build_nc()
    return _CACHED_NC


def _np_reference(queries, keys, values, q_padding_mask, key_padding_mask, attn_mask,
                  Wq, bq, Wk, bk, Wv, bv, Wo, bo):
    """Numpy fallback for non-trivial masks/biases (never hit by the harness)."""
    q = queries @ Wq.T + bq
    k = keys @ Wk.T + bk
    v = values @ Wv.T + bv
    q = q.reshape(B, T, H, D).transpose(0, 2, 1, 3)
    k = k.reshape(B, S, H, D).transpose(0, 2, 1, 3)
    v = v.reshape(B, S, H, D).transpose(0, 2, 1, 3)
    scores = np.einsum("bhtd,bhsd->bhts", q, k) / np.sqrt(D)
    scores = np.where(key_padding_mask[:, None, None, :], -1e30, scores)
    scores = np.where(attn_mask[:, None, :, :], scores, -1e30)
    scores = scores - scores.max(axis=-1, keepdims=True)
    e = np.exp(scores)
    weights = e / e.sum(axis=-1, keepdims=True)
    weights = np.where(q_padding_mask[:, None, :, None], 0.0, weights)
    context = np.einsum("bhts,bhsd->bhtd", weights, v)
    context = context.transpose(0, 2, 1, 3).reshape(B, T, E)
    out = context @ Wo.T + bo
    return out.astype(np.float32), weights.astype(np.float32)


def kernel(queries, keys, values, q_padding_mask, key_padding_mask, attn_mask,
           Wq, bq, Wk, bk, Wv, bv, Wo, bo, _trace=False):
    queries = np.asarray(queries, np.float32)
    keys = np.asarray(keys, np.float32)
    values = np.asarray(values, np.float32)
    q_padding_mask = np.asarray(q_padding_mask)
    key_padding_mask = np.asarray(key_padding_mask)
    attn_mask = np.asarray(attn_mask)
    Wq, bq = np.asarray(Wq, np.float32), np.asarray(bq, np.float32)
    Wk, bk = np.asarray(Wk, np.float32), np.asarray(bk, np.float32)
    Wv, bv = np.asarray(Wv, np.float32), np.asarray(bv, np.float32)
    Wo, bo = np.asarray(Wo, np.float32), np.asarray(bo, np.float32)

    trivial = (
        not q_padding_mask.any()
        and not key_padding_mask.any()
        and attn_mask.all()
        and not bq.any() and not bk.any() and not bv.any() and not bo.any()
    )
    if not trivial:
        return _np_reference(
            queries, keys, values, q_padding_mask, key_padding_mask, attn_mask,
            Wq, bq, Wk, bk, Wv, bv, Wo, bo,
        )

    nc = _get_nc()

    wqT = np.ascontiguousarray(Wq.T).astype(NPBF16)
    wkT = np.ascontiguousarray(Wk.T).astype(NPBF16)
    wvT = np.ascontiguousarray(Wv.T).astype(NPBF16)
    woT = np.ascontiguousarray(Wo.T).astype(NPBF16)
    in_maps = []
    for b in range(B):
        in_maps.append(
            {
                "qT_in": np.ascontiguousarray(queries[b].T).astype(NPBF16),
                "kT_in": np.ascontiguousarray(keys[b].T).astype(NPBF16),
                "vT_in": np.ascontiguousarray(values[b].T).astype(NPBF16),
                "wqT": wqT,
                "wkT": wkT,
                "wvT": wvT,
                "woT": woT,
            }
        )

    res = run_bass_kernel_spmd(nc, in_maps, core_ids=list(range(B)), trace=_trace)
    if _trace and res.exec_time_ns is not None:
        kernel.last_exec_time_ns = res.exec_time_ns

    out = np.stack([res.results[b]["out"] for b in range(B)])  # (B, T, E) f32
    w_dev = np.stack([res.results[b]["w"] for b in range(B)])  # (B, H, S, T) bf16
    weights = np.ascontiguousarray(w_dev.swapaxes(2, 3)).astype(np.float32)
    return out, weights
